# revision 1
# baseline (speedup 1.0000x reference)
"""ResNet BasicBlock (conv3x3-BN-ReLU-conv3x3-BN-add-ReLU) on 8 Trainium2 cores.

Strategy:
  - Pure data parallel: batch 32 -> 4 images per core; weights/BN replicated.
  - BN folded into conv weights on host (w *= gamma*rsqrt(var+eps); bias terms
    kept separate, applied on-chip per output-channel partition).
  - Conv3x3 = 9 shifted 1x1 convs = matmuls accumulated in PSUM:
      out[O, spatial] += wT[I(128part), O] @ x_shift[I(128part), spatial]
    with channels on partitions (256 ch = 2 blocks of 128), spatial chunked
    into 8 rows x 56 cols = 448 columns per PSUM bank.
  - Inputs padded to 58x58 on host (zeros), conv1 output padded on-chip, so
    shifted windows are plain strided APs.
  - fp16 matmul operands (same PE rate as bf16, ~8x better precision),
    fp32 PSUM accumulation, fp32 epilogues and output.
"""

import numpy as np

import concourse.mybir as mybir
import concourse.tile as tile
from concourse import bacc
from concourse.bass_utils import run_bass_kernel_spmd

EPS = 1e-5
NCORES = 8
N, C, H, W = 32, 256, 56, 56
NPC = N // NCORES          # images per core
HP, WP = H + 2, W + 2      # padded spatial
CB = C // 128              # channel blocks (2)
RC = 8                     # rows per PSUM chunk
NCHUNK = H // RC           # 7 chunks
F16 = mybir.dt.float16
F32 = mybir.dt.float32

_CACHE = {}


def _build():
    nc = bacc.Bacc("TRN2", target_bir_lowering=False, debug=False,
                   num_devices=NCORES)
    xp = nc.dram_tensor("xp", [NPC, CB, 128, HP, WP], F16,
                        kind="ExternalInput").ap()
    w1t = nc.dram_tensor("w1t", [CB, 128, 9, C], F16, kind="ExternalInput").ap()
    w2t = nc.dram_tensor("w2t", [CB, 128, 9, C], F16, kind="ExternalInput").ap()
    b1 = nc.dram_tensor("b1", [CB, 128, 1], F32, kind="ExternalInput").ap()
    b2 = nc.dram_tensor("b2", [CB, 128, 1], F32, kind="ExternalInput").ap()
    y = nc.dram_tensor("y", [NPC, CB, 128, H, W], F32,
                       kind="ExternalOutput").ap()

    Relu = mybir.ActivationFunctionType.Relu
    Add = mybir.AluOpType.add

    with tile.TileContext(nc) as tc:
        with tc.tile_pool(name="w", bufs=1) as wp, \
             tc.tile_pool(name="x", bufs=3) as xpool, \
             tc.tile_pool(name="h", bufs=1) as hpool, \
             tc.tile_pool(name="yst", bufs=2) as ypool, \
             tc.tile_pool(name="tmp", bufs=4) as tpool, \
             tc.tile_pool(name="ps", bufs=8, space="PSUM") as pspool:

            # Startup: DMA issues serialize at ~620ns each on the Sync queue,
            # so order by first-need. The first matmul group (ob=0, chunk 0)
            # needs only x0 rows 0:10 and the ob=0 half of w1; Tile tracks
            # subtile ranges, so finer pieces unblock the PE sooner.
            # conv2's weights aren't needed for ~100us -> DMA'd after conv1(0).
            hh = HP // 2
            w1s, w2s, b1s, b2s = [], [], [], []
            xt0 = [xpool.tile([128, HP, WP], F16, tag=f"x{ib}", name=f"xt0_{ib}")
                   for ib in range(CB)]
            for ib in range(CB):
                t = wp.tile([128, 9, C], F16, tag=f"w1_{ib}")
                w1s.append(t)
            for ib in range(CB):
                nc.sync.dma_start(out=xt0[ib][:, :10, :],
                                  in_=xp[0, ib, :, :10, :])
                nc.sync.dma_start(out=w1s[ib][:, :, :128],
                                  in_=w1t[ib, :, :, :128])
            for ib in range(CB):
                nc.sync.dma_start(out=xt0[ib][:, 10:hh, :],
                                  in_=xp[0, ib, :, 10:hh, :])
            for ib in range(CB):
                nc.sync.dma_start(out=xt0[ib][:, hh:, :],
                                  in_=xp[0, ib, :, hh:, :])
                nc.sync.dma_start(out=w1s[ib][:, :, 128:],
                                  in_=w1t[ib, :, :, 128:])
                t = wp.tile([128, 1], F32, tag=f"b1_{ib}")
                nc.sync.dma_start(out=t[:], in_=b1[ib])
                b1s.append(t)

            def load_w2():
                for ib in range(CB):
                    t = wp.tile([128, 9, C], F16, tag=f"w2_{ib}")
                    nc.sync.dma_start(out=t[:], in_=w2t[ib])
                    w2s.append(t)
                    t = wp.tile([128, 1], F32, tag=f"b2_{ib}")
                    nc.sync.dma_start(out=t[:], in_=b2[ib])
                    b2s.append(t)

            # PE warmup: the HAM clock gate holds the PE at 1.2 GHz until it
            # has been busy ~3.4us. The PE is idle during the initial DMA
            # wait anyway, so run throwaway matmuls on a zeroed scratch tile
            # to unthrottle the clock before the first real matmul.
            scratch = wp.tile([128, RC * W], F16, tag="warm_scratch")
            nc.gpsimd.memset(scratch[:], 0.0)
            # 16 cold matmuls: trips the ~3.4us HAM window AND keeps the PE
            # occupied until the second channel block's x/w DMAs have landed
            # (8 warmups measured 1.7us slower: the PE arrived early and
            # stalled on the ib=1 input DMA instead)
            ps_w = pspool.tile([128, RC * W], F32, name="ps_warm", tag="ps")
            for _ in range(16):
                nc.tensor.matmul(ps_w[:], scratch[:, :128], scratch[:],
                                 start=True, stop=True)

            # persistent conv1-output tiles: 2 channel blocks x 2 pipeline
            # parities. Fully zeroed ONCE here (so the 1-px border is zero
            # and no uninitialized element is ever read); the interior is
            # overwritten by conv1's epilogue every image.
            hts_all = {}
            for par in range(2):
                for ob in range(CB):
                    t = hpool.tile([128, HP, WP], F16, tag=f"h{par}_{ob}")
                    nc.vector.memset(t[:], 0.0)
                    hts_all[(par, ob)] = t

            def load_x(img):
                xt = []
                for ib in range(CB):
                    t = xpool.tile([128, HP, WP], F16, tag=f"x{ib}")
                    nc.sync.dma_start(out=t[:, :hh, :], in_=xp[img, ib, :, :hh, :])
                    nc.sync.dma_start(out=t[:, hh:, :], in_=xp[img, ib, :, hh:, :])
                    xt.append(t)
                return xt

            def conv1(img, xt):
                ht = [hts_all[(img % 2, ob)] for ob in range(CB)]
                for ob in range(CB):
                    for c in range(NCHUNK):
                        r0 = RC * c
                        ps = pspool.tile([128, RC, W], F32)
                        k = 0
                        for ib in range(CB):
                            for kx in range(3):
                                for ky in range(3):
                                    nc.tensor.matmul(
                                        ps[:],
                                        w1s[ib][:, 3 * ky + kx,
                                                128 * ob:128 * ob + 128],
                                        xt[ib][:, r0 + ky:r0 + ky + RC,
                                               kx:kx + W],
                                        start=(k == 0), stop=(k == 17))
                                    k += 1
                        nc.scalar.activation(
                            ht[ob][:, 1 + r0:1 + r0 + RC, 1:1 + W], ps[:],
                            Relu, bias=b1s[ob][:], scale=1.0)
                return ht

            def conv2(img, xt, ht):
                for ob in range(CB):
                    yt = ypool.tile([128, H, W], F32, tag=f"y{ob}")
                    # the very last group of the kernel sits on the critical
                    # path (MMs -> add -> relu -> DMA fully serial); split it
                    # into two half-height groups so the first half's
                    # epilogue overlaps the second half's matmuls
                    split_last = (img == NPC - 1 and ob == CB - 1)
                    groups = [(RC * c, RC) for c in range(NCHUNK - 1)]
                    if split_last:
                        groups += [(RC * (NCHUNK - 1), RC // 2),
                                   (RC * (NCHUNK - 1) + RC // 2, RC // 2)]
                    else:
                        groups += [(RC * (NCHUNK - 1), RC)]
                    pend = 0
                    for gi, (r0, nr) in enumerate(groups):
                        ps = pspool.tile([128, nr, W], F32, name="ps2",
                                         tag="ps")
                        k = 0
                        for ib in range(CB):
                            for kx in range(3):
                                for ky in range(3):
                                    nc.tensor.matmul(
                                        ps[:],
                                        w2s[ib][:, 3 * ky + kx,
                                                128 * ob:128 * ob + 128],
                                        ht[ib][:, r0 + ky:r0 + ky + nr,
                                               kx:kx + W],
                                        start=(k == 0), stop=(k == 17))
                                    k += 1
                        # residual add (identity = padded-x interior, fp16)
                        tmp = tpool.tile([128, nr, W], F32, name="tmp")
                        nc.vector.tensor_tensor(
                            out=tmp[:], in0=ps[:],
                            in1=xt[ob][:, 1 + r0:1 + r0 + nr, 1:1 + W],
                            op=Add)
                        # + per-channel bias, relu
                        nc.scalar.activation(
                            yt[:, r0:r0 + nr, :], tmp[:],
                            Relu, bias=b2s[ob][:], scale=1.0)
                        # stream the output out in row-group chunks so the
                        # final DMA isn't serialized after the last chunk
                        done = r0 + nr
                        if (done - pend >= 2 * RC or gi == len(groups) - 1
                                or (split_last and r0 >= RC * (NCHUNK - 1))):
                            nc.sync.dma_start(out=y[img, ob, :, pend:done, :],
                                              in_=yt[:, pend:done, :])
                            pend = done

            # software pipeline: conv1(i+1) emitted before conv2(i) so the PE
            # has independent work while conv2 waits on conv1's epilogue
            xts, hts = {}, {}
            xts[0] = xt0
            hts[0] = conv1(0, xts[0])
            load_w2()
            for img in range(1, NPC):
                xts[img] = load_x(img)
                hts[img] = conv1(img, xts[img])
                conv2(img - 1, xts[img - 1], hts[img - 1])
            conv2(NPC - 1, xts[NPC - 1], hts[NPC - 1])

    nc.compile()
    return nc


def _prep(inputs):
    x = np.asarray(inputs["x"], np.float32)
    out = {}
    for i in (1, 2):
        s = np.asarray(inputs[f"g{i}"], np.float32) / np.sqrt(
            np.asarray(inputs[f"rv{i}"], np.float32) + EPS)
        b = (np.asarray(inputs[f"b{i}"], np.float32)
             - np.asarray(inputs[f"rm{i}"], np.float32) * s)
        w = np.asarray(inputs[f"w{i}"], np.float32) * s[:, None, None, None]
        # [O,I,3,3] -> [I, ky, kx, O] -> [CB, 128, 9, O]
        wt = np.ascontiguousarray(w.transpose(1, 2, 3, 0)).reshape(
            C, 9, C).reshape(CB, 128, 9, C).astype(np.float16)
        out[f"w{i}t"] = wt
        out[f"b{i}"] = np.ascontiguousarray(b.reshape(CB, 128, 1))
    xpad = np.zeros((N, C, HP, WP), np.float16)
    xpad[:, :, 1:-1, 1:-1] = x
    out["xp"] = xpad.reshape(NCORES, NPC, CB, 128, HP, WP)
    return out


def run(inputs, trace=False):
    if "nc" not in _CACHE:
        _CACHE["nc"] = _build()
    nc = _CACHE["nc"]
    p = _prep(inputs)
    in_maps = [{"xp": p["xp"][c], "w1t": p["w1t"], "w2t": p["w2t"],
                "b1": p["b1"], "b2": p["b2"]} for c in range(NCORES)]
    res = run_bass_kernel_spmd(nc, in_maps, core_ids=list(range(NCORES)),
                               trace=trace)
    yout = np.concatenate(
        [r["y"].reshape(NPC, C, H, W) for r in res.results], axis=0)
    return yout, res


def kernel(**inputs):
    yout, _ = run(inputs)
    return yout

